# revision 21
# baseline (speedup 1.0000x reference)
"""Trainium2 Bass kernel for nn_Attention_29472065585724.

Reference computation (per batch b of 16, C=1024, H=W=32, seq p2=256, nh=8, hd=512):
    qkv = conv1x1(x, w_qkv, b_qkv)            # [B, 3C, H, W]
    q,k,v = reshape(B, 256, 3, 8, 512) ...    # row-major reshape mixing C and HW
    attn  = softmax(q @ k^T * scale) @ v
    out   = conv1x1(attn_reshaped, w_proj, b_proj)

Strategy (v14):
  - Data-parallel: batch 16 -> 8 cores x 2 batches. No collectives; host gathers.
  - ALL matmul operands bf16 (v13 used f32r for the big GEMMs). Measured on
    this HW: f32r matmuls pay a 227ns cadence per 512-col stream (weight
    reload not hidden - fp32 weights load in HI/LO passes, no FWL) while
    bf16 matmuls run at ~216ns (FWL hides the load). 1024 big-GEMM matmuls
    x 11ns = ~11us saved, and accuracy stays ~5e-3 « 2e-2 tolerance.
  - bf16 operands also kill the v13 bf16->f32r conversion stages whose
    Scalar/Vector latency gated the GEMM1 ramp (PE idled ~15us waiting on
    convert semaphores), and halve the w2/wp wire traffic.
  - GEMM1's first w1-quarter pass is k-outer (8 PSUM banks), consuming the
    (w1-quarter, x-half) DMA pairs in arrival order so the PE starts as soon
    as the first pair lands; all other GEMM phases are k-inner (back-to-back
    accumulation is ~75ns/matmul faster than bank-interleaved k-outer).
    Later w1 quarters stream into double-buffered slots behind the passes
    that consume them; w2/wp queue after the w1 quarters so they never
    steal ramp bandwidth.
  - w2/wp stay SBUF-resident across both batches; batch 1 replays batch 0's
    schedule into the same SBUF slots, refill ordering enforced by
    tile-reuse dependencies.
  - b1 is host-replicated to [128, 2048] f32 and DMA'd on the scalar queue.
  - Host-side weight permutation makes every device layout fall out of plain
    GEMMs with zero on-device transposes:
      * q,k produced transposed ([d, seq]) via x-stationary GEMM; softmax
        scale folded into w_q/b_q.
      * v produced in [seq, d]; proj contraction columns permuted so attention
        outputs land contiguously.
  - Softmax without max-subtraction (S bounded ~|6|); denominator via a tiny
    N=8 matmul of exp(S^T) against ones, normalization during PSUM eviction.
  - y stored bf16 and upcast on host.
"""
import sys

import numpy as np

if "/opt/trn_rl_repo" not in sys.path:
    sys.path.insert(0, "/opt/trn_rl_repo")

import ml_dtypes

import concourse.bass as bass
import concourse.tile as tile
from concourse import bacc, mybir
from concourse import bass_utils

F32 = mybir.dt.float32
BF16 = mybir.dt.bfloat16
AF = mybir.ActivationFunctionType
BF16_NP = ml_dtypes.bfloat16

B_PER_CORE = 2
N_CORES = 8
CIN = 1024
HW = 1024
NH = 8
P2 = 256
HD = 512

_CACHE = {}


def _build_program():
    nc = bacc.Bacc("TRN2", target_bir_lowering=False, debug=False)
    x_d = nc.dram_tensor("xf", [B_PER_CORE, CIN, HW], BF16,
                         kind="ExternalInput").ap()
    w1_d = nc.dram_tensor("w1q", [4, CIN, 512], BF16, kind="ExternalInput").ap()
    w2_d = nc.dram_tensor("w2t", [CIN, 1024], BF16, kind="ExternalInput").ap()
    wp_d = nc.dram_tensor("wpt", [1024, 1024], BF16, kind="ExternalInput").ap()
    b1_d = nc.dram_tensor("b1r", [128, 2048], F32, kind="ExternalInput").ap()
    b2_d = nc.dram_tensor("b2", [1024], F32, kind="ExternalInput").ap()
    bp_d = nc.dram_tensor("bp", [1024], F32, kind="ExternalInput").ap()
    ones_d = nc.dram_tensor("ones_c", [128, 8], BF16, kind="ExternalInput").ap()
    y_d = nc.dram_tensor("y", [B_PER_CORE, 1024, HW], BF16, kind="ExternalOutput").ap()

    with tile.TileContext(nc) as tc:
        with tile.ExitStack() as top:
            persist = top.enter_context(tc.tile_pool(name="persist", bufs=1))
            y_pool = top.enter_context(tc.tile_pool(name="ypool", bufs=4))
            w1_pool = top.enter_context(tc.tile_pool(name="w1pool", bufs=1))
            w2_pool = top.enter_context(tc.tile_pool(name="w2pool", bufs=1))

            # Inputs are split across BOTH hardware DGE queues: the per-queue
            # DMA issue rate is only ~230GB/s (each DMA_DIRECT2D instruction
            # paces with its bytes on the issuing engine), so x / w2 / wp /
            # b1_bc stream on the Activation (scalar) queue while the w1
            # quarters stream on the SP (sync) queue in parallel.
            b2_sb = persist.tile([128, 8], F32, name="b2_sb")
            bp_sb = persist.tile([128, 8], F32, name="bp_sb")
            ones_col = persist.tile([128, 8], BF16, name="ones_col")
            b1_bc = persist.tile([128, 2048], F32, name="b1_bc")

            w2_sb = [w2_pool.tile([128, 1024], BF16, name=f"w2sb{k}", tag=f"w2sb{k}")
                     for k in range(8)]
            wp_pool = top.enter_context(tc.tile_pool(name="wppool", bufs=1))
            wp_sb = [wp_pool.tile([128, 1024], BF16, name=f"wpsb{k}", tag=f"wpsb{k}")
                     for k in range(8)]

            # x double-buffered across batches (persistent tiles): batch 1's
            # input DMAs have no slot-reuse dependency, so they stream during
            # batch 0's compute instead of queueing behind its y stores.
            x_pool = top.enter_context(tc.tile_pool(name="xpool", bufs=1))
            x_sb_all = [[x_pool.tile([128, HW], BF16, name=f"xsb{b}_{k}",
                                     tag=f"xsb{b}_{k}") for k in range(8)]
                        for b in range(B_PER_CORE)]

            # All SBUF data pools are top-level and persistent: batch 1
            # re-allocates the same tags, so cross-batch reuse is enforced by
            # exact tile dependencies instead of pool-close barriers (a pool
            # close/reopen joins on ALL the pool's prior accesses and was
            # costing ~1us at each phase/batch boundary).
            qk_pool = top.enter_context(tc.tile_pool(name="qkpool", bufs=1))
            v_pool = top.enter_context(tc.tile_pool(name="vpool", bufs=1))
            ao_pool = top.enter_context(tc.tile_pool(name="aopool", bufs=1))
            e_pool = top.enter_context(tc.tile_pool(name="epool", bufs=2))
            r_pool = top.enter_context(tc.tile_pool(name="rpool", bufs=4))

            # PE warmup: dummy matmuls on a zeroed scratch tile while the
            # first input DMAs are in flight. Costs nothing (the PE would
            # idle anyway) and raises the PE p-state clock (0.65 -> 2.4GHz
            # after ~3us of continuous execution) before the real pass-0.
            # The warmup PSUM pool releases before ps_pool opens below.
            scratch = persist.tile([128, 256], BF16, name="warm_sb")
            nc.vector.memset(scratch[:], 0.0)
            with tc.tile_pool(name="warmps", bufs=1, space="PSUM") as wps:
                wtile = wps.tile([128, 256], F32, name="warm_ps")
                for _ in range(13):
                    nc.tensor.matmul(wtile[:], scratch[:, 0:128],
                                     scratch[:, 0:256], start=True, stop=True)

            # ONE persistent PSUM pool for every accumulation in the program:
            # all tiles share tag "ps" and rotate through the 8 banks, so
            # bank reuse is an exact 8-allocations-back tile dependency and
            # no PSUM pool is ever closed mid-program.
            ps_pool = top.enter_context(tc.tile_pool(name="pspool", bufs=8,
                                                     space="PSUM"))

            shared = dict(nc=nc, tc=tc, w1_d=w1_d, w2_d=w2_d, wp_d=wp_d,
                          y_d=y_d, w1_pool=w1_pool, w2_sb=w2_sb, wp_sb=wp_sb,
                          b1_bc=b1_bc, b2_sb=b2_sb, bp_sb=bp_sb,
                          ones_col=ones_col, y_pool=y_pool, ps_pool=ps_pool,
                          qk_pool=qk_pool, v_pool=v_pool, ao_pool=ao_pool,
                          e_pool=e_pool, r_pool=r_pool)

            early0 = _issue_early_dmas(nc, 0, x_d, w1_d, x_sb_all[0], w1_pool)
            # behind x on the scalar queue: b1_bc in 4 chunks (chunk n gates
            # only pass-n's eviction; a single 1MB DMA would land ~30us in
            # and stall the PE ~5us), then consts, then w2 (needed ~75us)
            # and wp (~105us)
            for n in range(4):
                nc.scalar.dma_start(b1_bc[:, 512 * n:512 * n + 512],
                                    b1_d[:, 512 * n:512 * n + 512])
            nc.scalar.dma_start(b2_sb[:], b2_d.rearrange("(t p) -> p t", p=128))
            nc.scalar.dma_start(bp_sb[:], bp_d.rearrange("(t p) -> p t", p=128))
            nc.scalar.dma_start(ones_col[:], ones_d[:])
            for k in range(8):
                nc.scalar.dma_start(w2_sb[k][:], w2_d[128 * k:128 * k + 128, :])
            for k in range(8):
                nc.scalar.dma_start(wp_sb[k][:], wp_d[128 * k:128 * k + 128, :])
            ctx0 = _emit_front(shared, 0, x_sb_all[0], early0)
            early1 = _issue_early_dmas(nc, 1, x_d, w1_d, x_sb_all[1], w1_pool)
            _emit_proj(shared, ctx0)
            ctx1 = _emit_front(shared, 1, x_sb_all[1], early1)
            _emit_proj(shared, ctx1)
    nc.compile()
    return nc


def _issue_early_dmas(nc, b, x_d, w1_d, x_sb, w1_pool):
    """Queue batch b's GEMM1 ramp DMAs: x tiles on the scalar queue, w1
    quarters 0-1 on the sync queue - the two streams run in parallel and
    pass 0 consumes (x[k], w1q0[k]) pairs in arrival order. x[0] loads in
    two halves so pass 0's first m-sweep starts one DMA-latency earlier."""
    nc.scalar.dma_start(x_sb[0][:, 0:512], x_d[b, 0:128, 0:512])
    nc.scalar.dma_start(x_sb[0][:, 512:1024], x_d[b, 0:128, 512:1024])
    for k in range(1, 8):
        nc.scalar.dma_start(x_sb[k][:], x_d[b, 128 * k:128 * k + 128, :])
    q0 = [w1_pool.tile([128, 512], BF16, name=f"w1q{b}_0_{k}",
                       tag=f"qbuf0_{k}") for k in range(8)]
    for k in range(8):
        nc.sync.dma_start(q0[k][:], w1_d[0, 128 * k:128 * k + 128, :])
    q1 = [w1_pool.tile([128, 512], BF16, name=f"w1q{b}_1_{k}",
                       tag=f"qbuf1_{k}") for k in range(8)]
    for k in range(8):
        nc.sync.dma_start(q1[k][:], w1_d[1, 128 * k:128 * k + 128, :])
    return q0, q1


def _emit_front(shared, b, x_sb, early):
    nc, tc = shared["nc"], shared["tc"]
    w1_d = shared["w1_d"]
    w1_pool, w2_sb = shared["w1_pool"], shared["w2_sb"]
    b1_bc, b2_sb, ones_col = shared["b1_bc"], shared["b2_sb"], shared["ones_col"]
    psp = shared["ps_pool"]
    e_pool, r_pool = shared["e_pool"], shared["r_pool"]

    def load_w1_quarter(n):
        w1q = [w1_pool.tile([128, 512], BF16, name=f"w1q{b}_{n}_{k}",
                            tag=f"qbuf{n % 2}_{k}") for k in range(8)]
        for k in range(8):
            nc.sync.dma_start(w1q[k][:], w1_d[n, 128 * k:128 * k + 128, :])
        return w1q

    qkT = [shared["qk_pool"].tile([128, 2048], BF16, name=f"qkT{b}_{m}",
                                  tag=f"qkT{m}") for m in range(8)]
    v_sb = [shared["v_pool"].tile([128, 1024], BF16, name=f"vsb{b}_{m}",
                                  tag=f"vsb{m}") for m in range(8)]

    # ---------------- QKV GEMMs ----------------
    q0, q1 = early
    w1quads = [q0, q1, load_w1_quarter(2), load_w1_quarter(3)]

    # GEMM1 (q,k): quarter pass 0 k-outer, consuming the (x[k], w1q0[k])
    # DMA pairs in arrival order; passes 1-3 k-inner (back-to-back
    # accumulation is ~75ns/matmul faster than bank-interleaved k-outer)
    pss = [psp.tile([128, 512], F32, name=f"psg1_{b}_0_{m}",
                    tag="ps") for m in range(8)]
    for k in range(8):
        for m in range(8):
            nc.tensor.matmul(
                pss[m][:],
                x_sb[k][:, 128 * m:128 * m + 128],
                w1quads[0][k][:],
                start=(k == 0), stop=(k == 7))
    for m in range(8):
        nc.vector.tensor_add(qkT[m][:, 0:512], pss[m][:],
                             b1_bc[:, 0:512])
    for n in range(1, 4):
        w1q = w1quads[n]
        for m in range(8):
            ps = psp.tile([128, 512], F32, name=f"psg1_{b}_{n}_{m}",
                          tag="ps")
            for k in range(8):
                nc.tensor.matmul(
                    ps[:],
                    x_sb[k][:, 128 * m:128 * m + 128],
                    w1q[k][:],
                    start=(k == 0), stop=(k == 7))
            nc.vector.tensor_add(qkT[m][:, 512 * n:512 * n + 512],
                                 ps[:], b1_bc[:, 512 * n:512 * n + 512])

    # GEMM2 (v): k-inner
    for m in range(8):
        for n in range(2):
            ps = psp.tile([128, 512], F32, name=f"psg2_{b}_{m}_{n}",
                          tag="ps")
            for k in range(8):
                nc.tensor.matmul(
                    ps[:],
                    w2_sb[k][:, 128 * m:128 * m + 128],
                    x_sb[k][:, 512 * n:512 * n + 512],
                    start=(k == 0), stop=(k == 7))
            nc.scalar.activation(v_sb[m][:, 512 * n:512 * n + 512],
                                 ps[:], AF.Identity, bias=b2_sb[:, m:m + 1])

    # ---------------- attention ----------------
    ao_sb = [shared["ao_pool"].tile([128, 1024], BF16, name=f"aosb{b}_{m}",
                                    tag=f"ao{m}") for m in range(8)]

    def attn_st(h):
        g, half = h // 2, h % 2
        base = 4 * half
        es = []
        for kt in range(2):
            ps = psp.tile([128, 256], F32, name=f"ps_st{b}_{h}_{kt}",
                          tag="ps")
            for d in range(4):
                nc.tensor.matmul(
                    ps[:],
                    qkT[base + d][:, (4 + g) * 256 + 128 * kt:
                                  (4 + g) * 256 + 128 * kt + 128],
                    qkT[base + d][:, g * 256:g * 256 + 256],
                    start=(d == 0), stop=(d == 3))
            e = e_pool.tile([128, 256], BF16, name=f"E{b}_{h}_{kt}",
                            tag=f"E{kt}")
            nc.scalar.activation(e[:], ps[:], AF.Exp)
            es.append(e)
        return es

    def attn_pv(h, es):
        g, half = h // 2, h % 2
        for qt in range(2):
            psO = psp.tile([128, 512], F32, name=f"psO{b}_{h}_{qt}", tag="ps")
            psL = psp.tile([128, 8], F32, name=f"psL{b}_{h}_{qt}", tag="ps")
            for kt in range(2):
                nc.tensor.matmul(
                    psO[:], es[kt][:, 128 * qt:128 * qt + 128],
                    v_sb[2 * g + kt][:, 512 * half:512 * half + 512],
                    start=(kt == 0), stop=(kt == 1))
                nc.tensor.matmul(
                    psL[:], es[kt][:, 128 * qt:128 * qt + 128],
                    ones_col[:, 0:8],
                    start=(kt == 0), stop=(kt == 1))
            r = r_pool.tile([128, 1], F32, name=f"r{b}_{h}_{qt}", tag="r")
            nc.vector.reciprocal(r[:], psL[:, 0:1])
            dst = ao_sb[2 * g + qt]
            nc.vector.tensor_scalar_mul(
                dst[:, 512 * half:512 * half + 512], psO[:], r[:])

    es_next = attn_st(0)
    for h in range(NH):
        es_cur = es_next
        es_next = attn_st(h + 1) if h + 1 < NH else None
        attn_pv(h, es_cur)
    return dict(b=b, ao_sb=ao_sb)


def _emit_proj(shared, ctx):
    # ---------------- proj GEMM: k-inner ----------------
    nc = shared["nc"]
    wp_sb, bp_sb = shared["wp_sb"], shared["bp_sb"]
    y_pool, y_d = shared["y_pool"], shared["y_d"]
    psp = shared["ps_pool"]
    b, ao_sb = ctx["b"], ctx["ao_sb"]
    for m in range(8):
        for n in range(2):
            ps = psp.tile([128, 512], F32, name=f"psp{b}_{m}_{n}",
                          tag="ps")
            for k in range(8):
                nc.tensor.matmul(
                    ps[:],
                    wp_sb[k][:, 128 * m:128 * m + 128],
                    ao_sb[k][:, 512 * n:512 * n + 512],
                    start=(k == 0), stop=(k == 7))
            y_sb = y_pool.tile([128, 512], BF16, name=f"ysb{b}_{m}_{n}",
                               tag="ysb")
            if m == 7 and n == 1:
                # tail: evict the last tile in 256+128+128 chunks alternating
                # engines, each chunk DMA'd (sync queue) as soon as it is
                # ready, so the final store is a 32KB transfer
                nc.scalar.activation(y_sb[:, 0:256], ps[:, 0:256],
                                     AF.Identity, bias=bp_sb[:, m:m + 1])
                nc.sync.dma_start(
                    y_d[b, 896:1024, 512:768], y_sb[:, 0:256])
                nc.vector.tensor_scalar_add(y_sb[:, 256:384],
                                            ps[:, 256:384],
                                            bp_sb[:, m:m + 1])
                nc.sync.dma_start(
                    y_d[b, 896:1024, 768:896], y_sb[:, 256:384])
                nc.scalar.activation(y_sb[:, 384:512], ps[:, 384:512],
                                     AF.Identity, bias=bp_sb[:, m:m + 1])
                nc.sync.dma_start(
                    y_d[b, 896:1024, 896:1024], y_sb[:, 384:512])
            else:
                if (2 * m + n) % 2 == 0:
                    nc.scalar.activation(y_sb[:], ps[:], AF.Identity,
                                         bias=bp_sb[:, m:m + 1])
                else:
                    nc.vector.tensor_scalar_add(y_sb[:], ps[:],
                                                bp_sb[:, m:m + 1])
                nc.sync.dma_start(
                    y_d[b, 128 * m:128 * m + 128, 512 * n:512 * n + 512],
                    y_sb[:])


def _prepare_host_inputs(w_qkv, b_qkv, w_proj):
    """Permute weights so device layouts need no transposes. See layout notes."""
    C = CIN
    scale = np.float32((C // NH) ** -0.5)
    g_i, p_i = np.meshgrid(np.arange(4), np.arange(256), indexing="ij")
    # GEMM1 columns: (t, g, p) -> channel 12p + 4t + g
    t_i, g2_i, p2_i = np.meshgrid(np.arange(2), np.arange(4), np.arange(256),
                                  indexing="ij")
    src1 = (12 * p2_i + 4 * t_i + g2_i).reshape(-1)
    w1 = w_qkv[src1, :].astype(np.float32).copy()
    b1 = b_qkv[src1].astype(np.float32).copy()
    w1[:1024] *= scale
    b1[:1024] *= scale
    w1t = np.ascontiguousarray(w1.T)                       # [1024, 2048]
    # GEMM2 rows: r = g*256 + p -> channel 12p + 8 + g
    src2 = (12 * p_i + 8 + g_i).reshape(-1)
    w2t = np.ascontiguousarray(w_qkv[src2, :].T.astype(np.float32))   # [1024, 1024]
    b2 = b_qkv[src2].astype(np.float32).copy()
    # proj contraction: c' = g*256 + p -> orig col 4p + g
    srcp = (4 * p_i + g_i).reshape(-1)
    wpt = np.ascontiguousarray(w_proj[:, srcp].T.astype(np.float32))  # [1024, 1024]
    return w1t, b1, w2t, b2, wpt


def kernel(x, w_qkv, b_qkv, w_proj, b_proj):
    if "nc" not in _CACHE:
        _CACHE["nc"] = _build_program()
    nc = _CACHE["nc"]

    x = np.asarray(x, dtype=np.float32)
    B = x.shape[0]
    xf = x.reshape(B, CIN, HW).astype(BF16_NP)
    w1t, b1, w2t, b2, wpt = _prepare_host_inputs(
        np.asarray(w_qkv, np.float32), np.asarray(b_qkv, np.float32),
        np.asarray(w_proj, np.float32))
    # w1 as four contiguous quarter tensors [4, 1024, 512]
    w1q = np.ascontiguousarray(
        w1t.reshape(CIN, 4, 512).transpose(1, 0, 2)).astype(BF16_NP)
    b1r = np.ascontiguousarray(np.tile(b1.reshape(1, 2048), (128, 1)))
    bp = np.asarray(b_proj, np.float32)
    ones_c = np.ones((128, 8), BF16_NP)
    w2tb = w2t.astype(BF16_NP)
    wptb = wpt.astype(BF16_NP)

    in_maps = []
    for c in range(N_CORES):
        in_maps.append({
            "xf": np.ascontiguousarray(xf[c * B_PER_CORE:(c + 1) * B_PER_CORE]),
            "w1q": w1q, "w2t": w2tb, "wpt": wptb,
            "b1r": b1r, "b2": b2, "bp": bp,
            "ones_c": ones_c,
        })
    res = bass_utils.run_bass_kernel_spmd(nc, in_maps, core_ids=list(range(N_CORES)))
    _CACHE["last_results"] = res
    y = np.concatenate([np.asarray(res.results[c]["y"], dtype=np.float32)
                        for c in range(N_CORES)], axis=0)
    return np.ascontiguousarray(y.reshape(B, CIN, 32, 32))


# revision 23
# speedup vs baseline: 1.0019x; 1.0019x over previous
"""Trainium2 Bass kernel for nn_Attention_29472065585724.

Reference computation (per batch b of 16, C=1024, H=W=32, seq p2=256, nh=8, hd=512):
    qkv = conv1x1(x, w_qkv, b_qkv)            # [B, 3C, H, W]
    q,k,v = reshape(B, 256, 3, 8, 512) ...    # row-major reshape mixing C and HW
    attn  = softmax(q @ k^T * scale) @ v
    out   = conv1x1(attn_reshaped, w_proj, b_proj)

Strategy (v14):
  - Data-parallel: batch 16 -> 8 cores x 2 batches. No collectives; host gathers.
  - ALL matmul operands bf16 (v13 used f32r for the big GEMMs). Measured on
    this HW: f32r matmuls pay a 227ns cadence per 512-col stream (weight
    reload not hidden - fp32 weights load in HI/LO passes, no FWL) while
    bf16 matmuls run at ~216ns (FWL hides the load). 1024 big-GEMM matmuls
    x 11ns = ~11us saved, and accuracy stays ~5e-3 « 2e-2 tolerance.
  - bf16 operands also kill the v13 bf16->f32r conversion stages whose
    Scalar/Vector latency gated the GEMM1 ramp (PE idled ~15us waiting on
    convert semaphores), and halve the w2/wp wire traffic.
  - GEMM1's first w1-quarter pass is k-outer (8 PSUM banks), consuming the
    (w1-quarter, x-half) DMA pairs in arrival order so the PE starts as soon
    as the first pair lands; all other GEMM phases are k-inner (back-to-back
    accumulation is ~75ns/matmul faster than bank-interleaved k-outer).
    Later w1 quarters stream into double-buffered slots behind the passes
    that consume them; w2/wp queue after the w1 quarters so they never
    steal ramp bandwidth.
  - w2/wp stay SBUF-resident across both batches; batch 1 replays batch 0's
    schedule into the same SBUF slots, refill ordering enforced by
    tile-reuse dependencies.
  - b1 is host-replicated to [128, 2048] f32 and DMA'd on the scalar queue.
  - Host-side weight permutation makes every device layout fall out of plain
    GEMMs with zero on-device transposes:
      * q,k produced transposed ([d, seq]) via x-stationary GEMM; softmax
        scale folded into w_q/b_q.
      * v produced in [seq, d]; proj contraction columns permuted so attention
        outputs land contiguously.
  - Softmax without max-subtraction (S bounded ~|6|); denominator via a tiny
    N=8 matmul of exp(S^T) against ones, normalization during PSUM eviction.
  - y stored bf16 and upcast on host.
"""
import sys

import numpy as np

if "/opt/trn_rl_repo" not in sys.path:
    sys.path.insert(0, "/opt/trn_rl_repo")

import ml_dtypes

import concourse.bass as bass
import concourse.tile as tile
from concourse import bacc, mybir
from concourse import bass_utils

F32 = mybir.dt.float32
BF16 = mybir.dt.bfloat16
AF = mybir.ActivationFunctionType
BF16_NP = ml_dtypes.bfloat16

B_PER_CORE = 2
N_CORES = 8
CIN = 1024
HW = 1024
NH = 8
P2 = 256
HD = 512

_CACHE = {}


def _build_program():
    nc = bacc.Bacc("TRN2", target_bir_lowering=False, debug=False)
    x_d = nc.dram_tensor("xf", [B_PER_CORE, CIN, HW], BF16,
                         kind="ExternalInput").ap()
    w1_d = nc.dram_tensor("w1q", [4, CIN, 512], BF16, kind="ExternalInput").ap()
    w2_d = nc.dram_tensor("w2t", [CIN, 1024], BF16, kind="ExternalInput").ap()
    wp_d = nc.dram_tensor("wpt", [1024, 1024], BF16, kind="ExternalInput").ap()
    b1_d = nc.dram_tensor("b1r", [128, 2048], F32, kind="ExternalInput").ap()
    b2_d = nc.dram_tensor("b2", [1024], F32, kind="ExternalInput").ap()
    bp_d = nc.dram_tensor("bp", [1024], F32, kind="ExternalInput").ap()
    ones_d = nc.dram_tensor("ones_c", [128, 8], BF16, kind="ExternalInput").ap()
    y_d = nc.dram_tensor("y", [B_PER_CORE, 1024, HW], BF16, kind="ExternalOutput").ap()

    with tile.TileContext(nc) as tc:
        with tile.ExitStack() as top:
            persist = top.enter_context(tc.tile_pool(name="persist", bufs=1))
            y_pool = top.enter_context(tc.tile_pool(name="ypool", bufs=4))
            w1_pool = top.enter_context(tc.tile_pool(name="w1pool", bufs=1))
            w2_pool = top.enter_context(tc.tile_pool(name="w2pool", bufs=1))

            # Inputs are split across BOTH hardware DGE queues: the per-queue
            # DMA issue rate is only ~230GB/s (each DMA_DIRECT2D instruction
            # paces with its bytes on the issuing engine), so x / w2 / wp /
            # b1_bc stream on the Activation (scalar) queue while the w1
            # quarters stream on the SP (sync) queue in parallel.
            b2_sb = persist.tile([128, 8], F32, name="b2_sb")
            bp_sb = persist.tile([128, 8], F32, name="bp_sb")
            ones_col = persist.tile([128, 8], BF16, name="ones_col")
            b1_bc = persist.tile([128, 2048], F32, name="b1_bc")

            w2_sb = [w2_pool.tile([128, 1024], BF16, name=f"w2sb{k}", tag=f"w2sb{k}")
                     for k in range(8)]
            wp_pool = top.enter_context(tc.tile_pool(name="wppool", bufs=1))
            wp_sb = [wp_pool.tile([128, 1024], BF16, name=f"wpsb{k}", tag=f"wpsb{k}")
                     for k in range(8)]

            # x double-buffered across batches (persistent tiles): batch 1's
            # input DMAs have no slot-reuse dependency, so they stream during
            # batch 0's compute instead of queueing behind its y stores.
            x_pool = top.enter_context(tc.tile_pool(name="xpool", bufs=1))
            x_sb_all = [[x_pool.tile([128, HW], BF16, name=f"xsb{b}_{k}",
                                     tag=f"xsb{b}_{k}") for k in range(8)]
                        for b in range(B_PER_CORE)]

            # All SBUF data pools are top-level and persistent: batch 1
            # re-allocates the same tags, so cross-batch reuse is enforced by
            # exact tile dependencies instead of pool-close barriers (a pool
            # close/reopen joins on ALL the pool's prior accesses and was
            # costing ~1us at each phase/batch boundary).
            qk_pool = top.enter_context(tc.tile_pool(name="qkpool", bufs=1))
            v_pool = top.enter_context(tc.tile_pool(name="vpool", bufs=1))
            ao_pool = top.enter_context(tc.tile_pool(name="aopool", bufs=1))
            e_pool = top.enter_context(tc.tile_pool(name="epool", bufs=2))
            r_pool = top.enter_context(tc.tile_pool(name="rpool", bufs=4))

            # PE warmup: dummy matmuls on a zeroed scratch tile while the
            # first input DMAs are in flight. Costs nothing (the PE would
            # idle anyway) and raises the PE p-state clock (0.65 -> 2.4GHz
            # after ~3us of continuous execution) before the real pass-0.
            # The warmup PSUM pool releases before ps_pool opens below.
            scratch = persist.tile([128, 256], BF16, name="warm_sb")
            nc.vector.memset(scratch[:], 0.0)
            with tc.tile_pool(name="warmps", bufs=1, space="PSUM") as wps:
                wtile = wps.tile([128, 256], F32, name="warm_ps")
                for _ in range(16):
                    nc.tensor.matmul(wtile[:], scratch[:, 0:128],
                                     scratch[:, 0:256], start=True, stop=True)

            # ONE persistent PSUM pool for every accumulation in the program:
            # all tiles share tag "ps" and rotate through the 8 banks, so
            # bank reuse is an exact 8-allocations-back tile dependency and
            # no PSUM pool is ever closed mid-program.
            ps_pool = top.enter_context(tc.tile_pool(name="pspool", bufs=8,
                                                     space="PSUM"))

            shared = dict(nc=nc, tc=tc, w1_d=w1_d, w2_d=w2_d, wp_d=wp_d,
                          y_d=y_d, w1_pool=w1_pool, w2_sb=w2_sb, wp_sb=wp_sb,
                          b1_bc=b1_bc, b2_sb=b2_sb, bp_sb=bp_sb,
                          ones_col=ones_col, y_pool=y_pool, ps_pool=ps_pool,
                          qk_pool=qk_pool, v_pool=v_pool, ao_pool=ao_pool,
                          e_pool=e_pool, r_pool=r_pool)

            early0 = _issue_early_dmas(nc, 0, x_d, w1_d, x_sb_all[0], w1_pool)
            # behind x on the scalar queue: b1_bc in 4 chunks (chunk n gates
            # only pass-n's eviction; a single 1MB DMA would land ~30us in
            # and stall the PE ~5us), then consts, then w2 (needed ~75us)
            # and wp (~105us)
            for n in range(4):
                nc.scalar.dma_start(b1_bc[:, 512 * n:512 * n + 512],
                                    b1_d[:, 512 * n:512 * n + 512])
            nc.scalar.dma_start(b2_sb[:], b2_d.rearrange("(t p) -> p t", p=128))
            nc.scalar.dma_start(bp_sb[:], bp_d.rearrange("(t p) -> p t", p=128))
            nc.scalar.dma_start(ones_col[:], ones_d[:])
            for k in range(8):
                nc.scalar.dma_start(w2_sb[k][:], w2_d[128 * k:128 * k + 128, :])
            for k in range(8):
                nc.scalar.dma_start(wp_sb[k][:], wp_d[128 * k:128 * k + 128, :])
            ctx0 = _emit_front(shared, 0, x_sb_all[0], early0)
            early1 = _issue_early_dmas(nc, 1, x_d, w1_d, x_sb_all[1], w1_pool)
            _emit_proj(shared, ctx0)
            ctx1 = _emit_front(shared, 1, x_sb_all[1], early1)
            _emit_proj(shared, ctx1)
    nc.compile()
    return nc


def _issue_early_dmas(nc, b, x_d, w1_d, x_sb, w1_pool):
    """Queue batch b's GEMM1 ramp DMAs: x tiles on the scalar queue, w1
    quarters 0-1 on the sync queue - the two streams run in parallel and
    pass 0 consumes (x[k], w1q0[k]) pairs in arrival order."""
    for k in range(8):
        nc.scalar.dma_start(x_sb[k][:], x_d[b, 128 * k:128 * k + 128, :])
    q0 = [w1_pool.tile([128, 512], BF16, name=f"w1q{b}_0_{k}",
                       tag=f"qbuf0_{k}") for k in range(8)]
    for k in range(8):
        nc.sync.dma_start(q0[k][:], w1_d[0, 128 * k:128 * k + 128, :])
    q1 = [w1_pool.tile([128, 512], BF16, name=f"w1q{b}_1_{k}",
                       tag=f"qbuf1_{k}") for k in range(8)]
    for k in range(8):
        nc.sync.dma_start(q1[k][:], w1_d[1, 128 * k:128 * k + 128, :])
    return q0, q1


def _emit_front(shared, b, x_sb, early):
    nc, tc = shared["nc"], shared["tc"]
    w1_d = shared["w1_d"]
    w1_pool, w2_sb = shared["w1_pool"], shared["w2_sb"]
    b1_bc, b2_sb, ones_col = shared["b1_bc"], shared["b2_sb"], shared["ones_col"]
    psp = shared["ps_pool"]
    e_pool, r_pool = shared["e_pool"], shared["r_pool"]

    def load_w1_quarter(n):
        w1q = [w1_pool.tile([128, 512], BF16, name=f"w1q{b}_{n}_{k}",
                            tag=f"qbuf{n % 2}_{k}") for k in range(8)]
        for k in range(8):
            nc.sync.dma_start(w1q[k][:], w1_d[n, 128 * k:128 * k + 128, :])
        return w1q

    qkT = [shared["qk_pool"].tile([128, 2048], BF16, name=f"qkT{b}_{m}",
                                  tag=f"qkT{m}") for m in range(8)]
    v_sb = [shared["v_pool"].tile([128, 1024], BF16, name=f"vsb{b}_{m}",
                                  tag=f"vsb{m}") for m in range(8)]

    # ---------------- QKV GEMMs ----------------
    q0, q1 = early
    w1quads = [q0, q1, load_w1_quarter(2), load_w1_quarter(3)]

    # GEMM1 (q,k): quarter pass 0 k-outer, consuming the (x[k], w1q0[k])
    # DMA pairs in arrival order; passes 1-3 k-inner (back-to-back
    # accumulation is ~75ns/matmul faster than bank-interleaved k-outer)
    pss = [psp.tile([128, 512], F32, name=f"psg1_{b}_0_{m}",
                    tag="ps") for m in range(8)]
    for k in range(8):
        for m in range(8):
            nc.tensor.matmul(
                pss[m][:],
                x_sb[k][:, 128 * m:128 * m + 128],
                w1quads[0][k][:],
                start=(k == 0), stop=(k == 7))
    for m in range(8):
        nc.vector.tensor_add(qkT[m][:, 0:512], pss[m][:],
                             b1_bc[:, 0:512])
    for n in range(1, 4):
        w1q = w1quads[n]
        for m in range(8):
            ps = psp.tile([128, 512], F32, name=f"psg1_{b}_{n}_{m}",
                          tag="ps")
            for k in range(8):
                nc.tensor.matmul(
                    ps[:],
                    x_sb[k][:, 128 * m:128 * m + 128],
                    w1q[k][:],
                    start=(k == 0), stop=(k == 7))
            nc.vector.tensor_add(qkT[m][:, 512 * n:512 * n + 512],
                                 ps[:], b1_bc[:, 512 * n:512 * n + 512])

    # GEMM2 (v): k-inner
    for m in range(8):
        for n in range(2):
            ps = psp.tile([128, 512], F32, name=f"psg2_{b}_{m}_{n}",
                          tag="ps")
            for k in range(8):
                nc.tensor.matmul(
                    ps[:],
                    w2_sb[k][:, 128 * m:128 * m + 128],
                    x_sb[k][:, 512 * n:512 * n + 512],
                    start=(k == 0), stop=(k == 7))
            nc.scalar.activation(v_sb[m][:, 512 * n:512 * n + 512],
                                 ps[:], AF.Identity, bias=b2_sb[:, m:m + 1])

    # ---------------- attention ----------------
    ao_sb = [shared["ao_pool"].tile([128, 1024], BF16, name=f"aosb{b}_{m}",
                                    tag=f"ao{m}") for m in range(8)]

    def attn_st(h):
        g, half = h // 2, h % 2
        base = 4 * half
        es = []
        for kt in range(2):
            ps = psp.tile([128, 256], F32, name=f"ps_st{b}_{h}_{kt}",
                          tag="ps")
            for d in range(4):
                nc.tensor.matmul(
                    ps[:],
                    qkT[base + d][:, (4 + g) * 256 + 128 * kt:
                                  (4 + g) * 256 + 128 * kt + 128],
                    qkT[base + d][:, g * 256:g * 256 + 256],
                    start=(d == 0), stop=(d == 3))
            e = e_pool.tile([128, 256], BF16, name=f"E{b}_{h}_{kt}",
                            tag=f"E{kt}")
            nc.scalar.activation(e[:], ps[:], AF.Exp)
            es.append(e)
        return es

    def attn_pv(h, es):
        g, half = h // 2, h % 2
        for qt in range(2):
            psO = psp.tile([128, 512], F32, name=f"psO{b}_{h}_{qt}", tag="ps")
            psL = psp.tile([128, 8], F32, name=f"psL{b}_{h}_{qt}", tag="ps")
            for kt in range(2):
                nc.tensor.matmul(
                    psO[:], es[kt][:, 128 * qt:128 * qt + 128],
                    v_sb[2 * g + kt][:, 512 * half:512 * half + 512],
                    start=(kt == 0), stop=(kt == 1))
                nc.tensor.matmul(
                    psL[:], es[kt][:, 128 * qt:128 * qt + 128],
                    ones_col[:, 0:8],
                    start=(kt == 0), stop=(kt == 1))
            r = r_pool.tile([128, 1], F32, name=f"r{b}_{h}_{qt}", tag="r")
            nc.vector.reciprocal(r[:], psL[:, 0:1])
            dst = ao_sb[2 * g + qt]
            nc.vector.tensor_scalar_mul(
                dst[:, 512 * half:512 * half + 512], psO[:], r[:])

    es_next = attn_st(0)
    for h in range(NH):
        es_cur = es_next
        es_next = attn_st(h + 1) if h + 1 < NH else None
        attn_pv(h, es_cur)
    return dict(b=b, ao_sb=ao_sb)


def _emit_proj(shared, ctx):
    # ---------------- proj GEMM: k-inner ----------------
    nc = shared["nc"]
    wp_sb, bp_sb = shared["wp_sb"], shared["bp_sb"]
    y_pool, y_d = shared["y_pool"], shared["y_d"]
    psp = shared["ps_pool"]
    b, ao_sb = ctx["b"], ctx["ao_sb"]
    for m in range(8):
        for n in range(2):
            ps = psp.tile([128, 512], F32, name=f"psp{b}_{m}_{n}",
                          tag="ps")
            for k in range(8):
                nc.tensor.matmul(
                    ps[:],
                    wp_sb[k][:, 128 * m:128 * m + 128],
                    ao_sb[k][:, 512 * n:512 * n + 512],
                    start=(k == 0), stop=(k == 7))
            y_sb = y_pool.tile([128, 512], BF16, name=f"ysb{b}_{m}_{n}",
                               tag="ysb")
            if m == 7 and n == 1:
                # tail: evict the last tile in 256+128+128 chunks alternating
                # engines, each chunk DMA'd (sync queue) as soon as it is
                # ready, so the final store is a 32KB transfer
                nc.scalar.activation(y_sb[:, 0:256], ps[:, 0:256],
                                     AF.Identity, bias=bp_sb[:, m:m + 1])
                nc.sync.dma_start(
                    y_d[b, 896:1024, 512:768], y_sb[:, 0:256])
                nc.vector.tensor_scalar_add(y_sb[:, 256:384],
                                            ps[:, 256:384],
                                            bp_sb[:, m:m + 1])
                nc.sync.dma_start(
                    y_d[b, 896:1024, 768:896], y_sb[:, 256:384])
                nc.scalar.activation(y_sb[:, 384:512], ps[:, 384:512],
                                     AF.Identity, bias=bp_sb[:, m:m + 1])
                nc.sync.dma_start(
                    y_d[b, 896:1024, 896:1024], y_sb[:, 384:512])
            else:
                if (2 * m + n) % 2 == 0:
                    nc.scalar.activation(y_sb[:], ps[:], AF.Identity,
                                         bias=bp_sb[:, m:m + 1])
                else:
                    nc.vector.tensor_scalar_add(y_sb[:], ps[:],
                                                bp_sb[:, m:m + 1])
                nc.sync.dma_start(
                    y_d[b, 128 * m:128 * m + 128, 512 * n:512 * n + 512],
                    y_sb[:])


def _prepare_host_inputs(w_qkv, b_qkv, w_proj):
    """Permute weights so device layouts need no transposes. See layout notes."""
    C = CIN
    scale = np.float32((C // NH) ** -0.5)
    g_i, p_i = np.meshgrid(np.arange(4), np.arange(256), indexing="ij")
    # GEMM1 columns: (t, g, p) -> channel 12p + 4t + g
    t_i, g2_i, p2_i = np.meshgrid(np.arange(2), np.arange(4), np.arange(256),
                                  indexing="ij")
    src1 = (12 * p2_i + 4 * t_i + g2_i).reshape(-1)
    w1 = w_qkv[src1, :].astype(np.float32).copy()
    b1 = b_qkv[src1].astype(np.float32).copy()
    w1[:1024] *= scale
    b1[:1024] *= scale
    w1t = np.ascontiguousarray(w1.T)                       # [1024, 2048]
    # GEMM2 rows: r = g*256 + p -> channel 12p + 8 + g
    src2 = (12 * p_i + 8 + g_i).reshape(-1)
    w2t = np.ascontiguousarray(w_qkv[src2, :].T.astype(np.float32))   # [1024, 1024]
    b2 = b_qkv[src2].astype(np.float32).copy()
    # proj contraction: c' = g*256 + p -> orig col 4p + g
    srcp = (4 * p_i + g_i).reshape(-1)
    wpt = np.ascontiguousarray(w_proj[:, srcp].T.astype(np.float32))  # [1024, 1024]
    return w1t, b1, w2t, b2, wpt


def kernel(x, w_qkv, b_qkv, w_proj, b_proj):
    if "nc" not in _CACHE:
        _CACHE["nc"] = _build_program()
    nc = _CACHE["nc"]

    x = np.asarray(x, dtype=np.float32)
    B = x.shape[0]
    xf = x.reshape(B, CIN, HW).astype(BF16_NP)
    w1t, b1, w2t, b2, wpt = _prepare_host_inputs(
        np.asarray(w_qkv, np.float32), np.asarray(b_qkv, np.float32),
        np.asarray(w_proj, np.float32))
    # w1 as four contiguous quarter tensors [4, 1024, 512]
    w1q = np.ascontiguousarray(
        w1t.reshape(CIN, 4, 512).transpose(1, 0, 2)).astype(BF16_NP)
    b1r = np.ascontiguousarray(np.tile(b1.reshape(1, 2048), (128, 1)))
    bp = np.asarray(b_proj, np.float32)
    ones_c = np.ones((128, 8), BF16_NP)
    w2tb = w2t.astype(BF16_NP)
    wptb = wpt.astype(BF16_NP)

    in_maps = []
    for c in range(N_CORES):
        in_maps.append({
            "xf": np.ascontiguousarray(xf[c * B_PER_CORE:(c + 1) * B_PER_CORE]),
            "w1q": w1q, "w2t": w2tb, "wpt": wptb,
            "b1r": b1r, "b2": b2, "bp": bp,
            "ones_c": ones_c,
        })
    res = bass_utils.run_bass_kernel_spmd(nc, in_maps, core_ids=list(range(N_CORES)))
    _CACHE["last_results"] = res
    y = np.concatenate([np.asarray(res.results[c]["y"], dtype=np.float32)
                        for c in range(N_CORES)], axis=0)
    return np.ascontiguousarray(y.reshape(B, CIN, 32, 32))


# revision 25
# speedup vs baseline: 1.0195x; 1.0176x over previous
"""Trainium2 Bass kernel for nn_Attention_29472065585724.

Reference computation (per batch b of 16, C=1024, H=W=32, seq p2=256, nh=8, hd=512):
    qkv = conv1x1(x, w_qkv, b_qkv)            # [B, 3C, H, W]
    q,k,v = reshape(B, 256, 3, 8, 512) ...    # row-major reshape mixing C and HW
    attn  = softmax(q @ k^T * scale) @ v
    out   = conv1x1(attn_reshaped, w_proj, b_proj)

Strategy (v14):
  - Data-parallel: batch 16 -> 8 cores x 2 batches. No collectives; host gathers.
  - ALL matmul operands bf16 (v13 used f32r for the big GEMMs). Measured on
    this HW: f32r matmuls pay a 227ns cadence per 512-col stream (weight
    reload not hidden - fp32 weights load in HI/LO passes, no FWL) while
    bf16 matmuls run at ~216ns (FWL hides the load). 1024 big-GEMM matmuls
    x 11ns = ~11us saved, and accuracy stays ~5e-3 « 2e-2 tolerance.
  - bf16 operands also kill the v13 bf16->f32r conversion stages whose
    Scalar/Vector latency gated the GEMM1 ramp (PE idled ~15us waiting on
    convert semaphores), and halve the w2/wp wire traffic.
  - GEMM1's first w1-quarter pass is k-outer (8 PSUM banks), consuming the
    (w1-quarter, x-half) DMA pairs in arrival order so the PE starts as soon
    as the first pair lands; all other GEMM phases are k-inner (back-to-back
    accumulation is ~75ns/matmul faster than bank-interleaved k-outer).
    Later w1 quarters stream into double-buffered slots behind the passes
    that consume them; w2/wp queue after the w1 quarters so they never
    steal ramp bandwidth.
  - w2/wp stay SBUF-resident across both batches; batch 1 replays batch 0's
    schedule into the same SBUF slots, refill ordering enforced by
    tile-reuse dependencies.
  - b1 is host-replicated to [128, 2048] f32 and DMA'd on the scalar queue.
  - Host-side weight permutation makes every device layout fall out of plain
    GEMMs with zero on-device transposes:
      * q,k produced transposed ([d, seq]) via x-stationary GEMM; softmax
        scale folded into w_q/b_q.
      * v produced in [seq, d]; proj contraction columns permuted so attention
        outputs land contiguously.
  - Softmax without max-subtraction (S bounded ~|6|); denominator via a tiny
    N=8 matmul of exp(S^T) against ones, normalization during PSUM eviction.
  - y stored bf16 and upcast on host.
"""
import sys

import numpy as np

if "/opt/trn_rl_repo" not in sys.path:
    sys.path.insert(0, "/opt/trn_rl_repo")

import ml_dtypes

import concourse.bass as bass
import concourse.tile as tile
from concourse import bacc, mybir
from concourse import bass_utils

F32 = mybir.dt.float32
BF16 = mybir.dt.bfloat16
AF = mybir.ActivationFunctionType
BF16_NP = ml_dtypes.bfloat16

B_PER_CORE = 2
N_CORES = 8
CIN = 1024
HW = 1024
NH = 8
P2 = 256
HD = 512

_CACHE = {}


def _build_program():
    nc = bacc.Bacc("TRN2", target_bir_lowering=False, debug=False)
    x_d = nc.dram_tensor("xf", [B_PER_CORE, CIN, HW], BF16,
                         kind="ExternalInput").ap()
    w1_d = nc.dram_tensor("w1q", [4, CIN, 512], BF16, kind="ExternalInput").ap()
    w2_d = nc.dram_tensor("w2t", [CIN, 1024], BF16, kind="ExternalInput").ap()
    wp_d = nc.dram_tensor("wpt", [1024, 1024], BF16, kind="ExternalInput").ap()
    b1_d = nc.dram_tensor("b1r", [128, 2048], F32, kind="ExternalInput").ap()
    b2_d = nc.dram_tensor("b2", [1024], F32, kind="ExternalInput").ap()
    bp_d = nc.dram_tensor("bp", [1024], F32, kind="ExternalInput").ap()
    ones_d = nc.dram_tensor("ones_c", [128, 8], BF16, kind="ExternalInput").ap()
    y_d = nc.dram_tensor("y", [B_PER_CORE, 1024, HW], BF16, kind="ExternalOutput").ap()

    with tile.TileContext(nc) as tc:
        with tile.ExitStack() as top:
            persist = top.enter_context(tc.tile_pool(name="persist", bufs=1))
            y_pool = top.enter_context(tc.tile_pool(name="ypool", bufs=4))
            w1_pool = top.enter_context(tc.tile_pool(name="w1pool", bufs=1))
            w2_pool = top.enter_context(tc.tile_pool(name="w2pool", bufs=1))

            # Inputs are split across BOTH hardware DGE queues: the per-queue
            # DMA issue rate is only ~230GB/s (each DMA_DIRECT2D instruction
            # paces with its bytes on the issuing engine), so x / w2 / wp /
            # b1_bc stream on the Activation (scalar) queue while the w1
            # quarters stream on the SP (sync) queue in parallel.
            b2_sb = persist.tile([128, 8], F32, name="b2_sb")
            bp_sb = persist.tile([128, 8], F32, name="bp_sb")
            ones_col = persist.tile([128, 8], BF16, name="ones_col")
            b1_bc = persist.tile([128, 2048], F32, name="b1_bc")

            w2_sb = [w2_pool.tile([128, 1024], BF16, name=f"w2sb{k}", tag=f"w2sb{k}")
                     for k in range(8)]
            wp_pool = top.enter_context(tc.tile_pool(name="wppool", bufs=1))
            wp_sb = [wp_pool.tile([128, 1024], BF16, name=f"wpsb{k}", tag=f"wpsb{k}")
                     for k in range(8)]

            # x double-buffered across batches (persistent tiles): batch 1's
            # input DMAs have no slot-reuse dependency, so they stream during
            # batch 0's compute instead of queueing behind its y stores.
            x_pool = top.enter_context(tc.tile_pool(name="xpool", bufs=1))
            x_sb_all = [[x_pool.tile([128, HW], BF16, name=f"xsb{b}_{k}",
                                     tag=f"xsb{b}_{k}") for k in range(8)]
                        for b in range(B_PER_CORE)]

            # All SBUF data pools are top-level and persistent: batch 1
            # re-allocates the same tags, so cross-batch reuse is enforced by
            # exact tile dependencies instead of pool-close barriers (a pool
            # close/reopen joins on ALL the pool's prior accesses and was
            # costing ~1us at each phase/batch boundary).
            qk_pool = top.enter_context(tc.tile_pool(name="qkpool", bufs=1))
            v_pool = top.enter_context(tc.tile_pool(name="vpool", bufs=1))
            ao_pool = top.enter_context(tc.tile_pool(name="aopool", bufs=1))
            e_pool = top.enter_context(tc.tile_pool(name="epool", bufs=2))
            r_pool = top.enter_context(tc.tile_pool(name="rpool", bufs=4))

            # PE warmup: dummy matmuls on a zeroed scratch tile while the
            # first input DMAs are in flight. Costs nothing (the PE would
            # idle anyway) and raises the PE p-state clock (0.65 -> 2.4GHz
            # after ~3us of continuous execution) before the real pass-0.
            # The warmup PSUM pool releases before ps_pool opens below.
            scratch = persist.tile([128, 256], BF16, name="warm_sb")
            nc.vector.memset(scratch[:], 0.0)
            with tc.tile_pool(name="warmps", bufs=1, space="PSUM") as wps:
                wtile = wps.tile([128, 256], F32, name="warm_ps")
                for _ in range(16):
                    nc.tensor.matmul(wtile[:], scratch[:, 0:128],
                                     scratch[:, 0:256], start=True, stop=True)

            # ONE persistent PSUM pool for every accumulation in the program:
            # all tiles share tag "ps" and rotate through the 8 banks, so
            # bank reuse is an exact 8-allocations-back tile dependency and
            # no PSUM pool is ever closed mid-program.
            ps_pool = top.enter_context(tc.tile_pool(name="pspool", bufs=8,
                                                     space="PSUM"))

            shared = dict(nc=nc, tc=tc, w1_d=w1_d, w2_d=w2_d, wp_d=wp_d,
                          y_d=y_d, w1_pool=w1_pool, w2_sb=w2_sb, wp_sb=wp_sb,
                          b1_bc=b1_bc, b2_sb=b2_sb, bp_sb=bp_sb,
                          ones_col=ones_col, y_pool=y_pool, ps_pool=ps_pool,
                          qk_pool=qk_pool, v_pool=v_pool, ao_pool=ao_pool,
                          e_pool=e_pool, r_pool=r_pool)

            early0 = _issue_early_dmas(nc, 0, x_d, w1_d, x_sb_all[0], w1_pool)
            # behind x on the scalar queue: b1_bc in 4 chunks (chunk n gates
            # only pass-n's eviction; a single 1MB DMA would land ~30us in
            # and stall the PE ~5us), then consts, then w2 (needed ~75us)
            # and wp (~105us)
            for n in range(4):
                nc.scalar.dma_start(b1_bc[:, 512 * n:512 * n + 512],
                                    b1_d[:, 512 * n:512 * n + 512])
            nc.scalar.dma_start(b2_sb[:], b2_d.rearrange("(t p) -> p t", p=128))
            nc.scalar.dma_start(bp_sb[:], bp_d.rearrange("(t p) -> p t", p=128))
            nc.scalar.dma_start(ones_col[:], ones_d[:])
            for k in range(8):
                nc.scalar.dma_start(w2_sb[k][:], w2_d[128 * k:128 * k + 128, :])
            for k in range(8):
                nc.scalar.dma_start(wp_sb[k][:], wp_d[128 * k:128 * k + 128, :])
            ctx0 = _emit_front(shared, 0, x_sb_all[0], early0)
            early1 = _issue_early_dmas(nc, 1, x_d, w1_d, x_sb_all[1], w1_pool)
            _emit_proj(shared, ctx0)
            ctx1 = _emit_front(shared, 1, x_sb_all[1], early1)
            _emit_proj(shared, ctx1)
    nc.compile()
    return nc


def _issue_early_dmas(nc, b, x_d, w1_d, x_sb, w1_pool):
    """Queue batch b's GEMM1 ramp DMAs: x tiles on the scalar queue, w1
    quarters 0-1 on the sync queue - the two streams run in parallel and
    pass 0 consumes (x[k], w1q0[k]) pairs in arrival order."""
    for k in range(8):
        nc.scalar.dma_start(x_sb[k][:], x_d[b, 128 * k:128 * k + 128, :])
    q0 = [w1_pool.tile([128, 512], BF16, name=f"w1q{b}_0_{k}",
                       tag=f"qbuf0_{k}") for k in range(8)]
    for k in range(8):
        nc.sync.dma_start(q0[k][:], w1_d[0, 128 * k:128 * k + 128, :])
    q1 = [w1_pool.tile([128, 512], BF16, name=f"w1q{b}_1_{k}",
                       tag=f"qbuf1_{k}") for k in range(8)]
    for k in range(8):
        nc.sync.dma_start(q1[k][:], w1_d[1, 128 * k:128 * k + 128, :])
    return q0, q1


def _emit_front(shared, b, x_sb, early):
    nc, tc = shared["nc"], shared["tc"]
    w1_d = shared["w1_d"]
    w1_pool, w2_sb = shared["w1_pool"], shared["w2_sb"]
    b1_bc, b2_sb, ones_col = shared["b1_bc"], shared["b2_sb"], shared["ones_col"]
    psp = shared["ps_pool"]
    e_pool, r_pool = shared["e_pool"], shared["r_pool"]

    def load_w1_quarter(n):
        w1q = [w1_pool.tile([128, 512], BF16, name=f"w1q{b}_{n}_{k}",
                            tag=f"qbuf{n % 2}_{k}") for k in range(8)]
        for k in range(8):
            nc.sync.dma_start(w1q[k][:], w1_d[n, 128 * k:128 * k + 128, :])
        return w1q

    qkT = [shared["qk_pool"].tile([128, 2048], BF16, name=f"qkT{b}_{m}",
                                  tag=f"qkT{m}") for m in range(8)]
    v_sb = [shared["v_pool"].tile([128, 1024], BF16, name=f"vsb{b}_{m}",
                                  tag=f"vsb{m}") for m in range(8)]

    # ---------------- QKV GEMMs ----------------
    q0, q1 = early
    w1quads = [q0, q1, load_w1_quarter(2), load_w1_quarter(3)]

    # GEMM1 (q,k): quarter pass 0 k-outer, consuming the (x[k], w1q0[k])
    # DMA pairs in arrival order; passes 1-3 k-inner (back-to-back
    # accumulation is ~75ns/matmul faster than bank-interleaved k-outer)
    pss = [psp.tile([128, 512], F32, name=f"psg1_{b}_0_{m}",
                    tag="ps") for m in range(8)]
    for k in range(8):
        for m in range(8):
            nc.tensor.matmul(
                pss[m][:],
                x_sb[k][:, 128 * m:128 * m + 128],
                w1quads[0][k][:],
                start=(k == 0), stop=(k == 7))
    for m in range(8):
        nc.vector.tensor_add(qkT[m][:, 0:512], pss[m][:],
                             b1_bc[:, 0:512])
    for n in range(1, 4):
        w1q = w1quads[n]
        for m in range(8):
            ps = psp.tile([128, 512], F32, name=f"psg1_{b}_{n}_{m}",
                          tag="ps")
            for k in range(8):
                nc.tensor.matmul(
                    ps[:],
                    x_sb[k][:, 128 * m:128 * m + 128],
                    w1q[k][:],
                    start=(k == 0), stop=(k == 7))
            nc.vector.tensor_add(qkT[m][:, 512 * n:512 * n + 512],
                                 ps[:], b1_bc[:, 512 * n:512 * n + 512])

    # GEMM2 (v): k-inner
    for m in range(8):
        for n in range(2):
            ps = psp.tile([128, 512], F32, name=f"psg2_{b}_{m}_{n}",
                          tag="ps")
            for k in range(8):
                nc.tensor.matmul(
                    ps[:],
                    w2_sb[k][:, 128 * m:128 * m + 128],
                    x_sb[k][:, 512 * n:512 * n + 512],
                    start=(k == 0), stop=(k == 7))
            nc.scalar.activation(v_sb[m][:, 512 * n:512 * n + 512],
                                 ps[:], AF.Identity, bias=b2_sb[:, m:m + 1])

    # ---------------- attention ----------------
    ao_sb = [shared["ao_pool"].tile([128, 1024], BF16, name=f"aosb{b}_{m}",
                                    tag=f"ao{m}") for m in range(8)]

    def attn_st(h):
        g, half = h // 2, h % 2
        base = 4 * half
        es = []
        for kt in range(2):
            ps = psp.tile([128, 256], F32, name=f"ps_st{b}_{h}_{kt}",
                          tag="ps")
            for d in range(4):
                nc.tensor.matmul(
                    ps[:],
                    qkT[base + d][:, (4 + g) * 256 + 128 * kt:
                                  (4 + g) * 256 + 128 * kt + 128],
                    qkT[base + d][:, g * 256:g * 256 + 256],
                    start=(d == 0), stop=(d == 3))
            e = e_pool.tile([128, 256], BF16, name=f"E{b}_{h}_{kt}",
                            tag=f"E{kt}")
            nc.scalar.activation(e[:], ps[:], AF.Exp)
            es.append(e)
        return es

    def attn_pv(h, es):
        g, half = h // 2, h % 2
        for qt in range(2):
            psO = psp.tile([128, 512], F32, name=f"psO{b}_{h}_{qt}", tag="ps")
            psL = psp.tile([128, 8], F32, name=f"psL{b}_{h}_{qt}", tag="ps")
            for kt in range(2):
                nc.tensor.matmul(
                    psO[:], es[kt][:, 128 * qt:128 * qt + 128],
                    v_sb[2 * g + kt][:, 512 * half:512 * half + 512],
                    start=(kt == 0), stop=(kt == 1))
                nc.tensor.matmul(
                    psL[:], es[kt][:, 128 * qt:128 * qt + 128],
                    ones_col[:, 0:8],
                    start=(kt == 0), stop=(kt == 1))
            r = r_pool.tile([128, 1], F32, name=f"r{b}_{h}_{qt}", tag="r")
            nc.vector.reciprocal(r[:], psL[:, 0:1])
            dst = ao_sb[2 * g + qt]
            nc.vector.tensor_scalar_mul(
                dst[:, 512 * half:512 * half + 512], psO[:], r[:])

    es_next = attn_st(0)
    for h in range(NH):
        es_cur = es_next
        es_next = attn_st(h + 1) if h + 1 < NH else None
        attn_pv(h, es_cur)
    return dict(b=b, ao_sb=ao_sb)


def _emit_proj(shared, ctx):
    # ---------------- proj GEMM: k-inner ----------------
    nc = shared["nc"]
    wp_sb, bp_sb = shared["wp_sb"], shared["bp_sb"]
    y_pool, y_d = shared["y_pool"], shared["y_d"]
    psp = shared["ps_pool"]
    b, ao_sb = ctx["b"], ctx["ao_sb"]
    for m in range(8):
        for n in range(2):
            if b == 1 and m == 7 and n == 1:
                # final tile of the program: two 256-col accumulation groups
                # so the first half's store overlaps the second half's
                # matmuls and the very last store is only 64KB
                for c in range(2):
                    ps = psp.tile([128, 256], F32, name=f"pspf_{c}", tag="ps")
                    for k in range(8):
                        nc.tensor.matmul(
                            ps[:],
                            wp_sb[k][:, 896:1024],
                            ao_sb[k][:, 512 + 256 * c:768 + 256 * c],
                            start=(k == 0), stop=(k == 7))
                    y_sb = y_pool.tile([128, 256], BF16, name=f"ysbf_{c}",
                                       tag="ysbf")
                    if c == 0:
                        nc.scalar.activation(y_sb[:], ps[:], AF.Identity,
                                             bias=bp_sb[:, 7:8])
                    else:
                        nc.vector.tensor_scalar_add(y_sb[:], ps[:],
                                                    bp_sb[:, 7:8])
                    nc.sync.dma_start(
                        y_d[b, 896:1024, 512 + 256 * c:768 + 256 * c],
                        y_sb[:])
                continue
            ps = psp.tile([128, 512], F32, name=f"psp{b}_{m}_{n}",
                          tag="ps")
            for k in range(8):
                nc.tensor.matmul(
                    ps[:],
                    wp_sb[k][:, 128 * m:128 * m + 128],
                    ao_sb[k][:, 512 * n:512 * n + 512],
                    start=(k == 0), stop=(k == 7))
            y_sb = y_pool.tile([128, 512], BF16, name=f"ysb{b}_{m}_{n}",
                               tag="ysb")
            if m == 7 and n == 1:
                # tail: split the last eviction across both engines and
                # two DMAs so the final y store starts ~0.7us earlier
                nc.scalar.activation(y_sb[:, 0:256], ps[:, 0:256],
                                     AF.Identity, bias=bp_sb[:, m:m + 1])
                nc.sync.dma_start(
                    y_d[b, 896:1024, 512:768], y_sb[:, 0:256])
                nc.vector.tensor_scalar_add(y_sb[:, 256:512],
                                            ps[:, 256:512],
                                            bp_sb[:, m:m + 1])
                nc.sync.dma_start(
                    y_d[b, 896:1024, 768:1024], y_sb[:, 256:512])
            else:
                if (2 * m + n) % 2 == 0:
                    nc.scalar.activation(y_sb[:], ps[:], AF.Identity,
                                         bias=bp_sb[:, m:m + 1])
                else:
                    nc.vector.tensor_scalar_add(y_sb[:], ps[:],
                                                bp_sb[:, m:m + 1])
                nc.sync.dma_start(
                    y_d[b, 128 * m:128 * m + 128, 512 * n:512 * n + 512],
                    y_sb[:])


def _prepare_host_inputs(w_qkv, b_qkv, w_proj):
    """Permute weights so device layouts need no transposes. See layout notes."""
    C = CIN
    scale = np.float32((C // NH) ** -0.5)
    g_i, p_i = np.meshgrid(np.arange(4), np.arange(256), indexing="ij")
    # GEMM1 columns: (t, g, p) -> channel 12p + 4t + g
    t_i, g2_i, p2_i = np.meshgrid(np.arange(2), np.arange(4), np.arange(256),
                                  indexing="ij")
    src1 = (12 * p2_i + 4 * t_i + g2_i).reshape(-1)
    w1 = w_qkv[src1, :].astype(np.float32).copy()
    b1 = b_qkv[src1].astype(np.float32).copy()
    w1[:1024] *= scale
    b1[:1024] *= scale
    w1t = np.ascontiguousarray(w1.T)                       # [1024, 2048]
    # GEMM2 rows: r = g*256 + p -> channel 12p + 8 + g
    src2 = (12 * p_i + 8 + g_i).reshape(-1)
    w2t = np.ascontiguousarray(w_qkv[src2, :].T.astype(np.float32))   # [1024, 1024]
    b2 = b_qkv[src2].astype(np.float32).copy()
    # proj contraction: c' = g*256 + p -> orig col 4p + g
    srcp = (4 * p_i + g_i).reshape(-1)
    wpt = np.ascontiguousarray(w_proj[:, srcp].T.astype(np.float32))  # [1024, 1024]
    return w1t, b1, w2t, b2, wpt


def kernel(x, w_qkv, b_qkv, w_proj, b_proj):
    if "nc" not in _CACHE:
        _CACHE["nc"] = _build_program()
    nc = _CACHE["nc"]

    x = np.asarray(x, dtype=np.float32)
    B = x.shape[0]
    xf = x.reshape(B, CIN, HW).astype(BF16_NP)
    w1t, b1, w2t, b2, wpt = _prepare_host_inputs(
        np.asarray(w_qkv, np.float32), np.asarray(b_qkv, np.float32),
        np.asarray(w_proj, np.float32))
    # w1 as four contiguous quarter tensors [4, 1024, 512]
    w1q = np.ascontiguousarray(
        w1t.reshape(CIN, 4, 512).transpose(1, 0, 2)).astype(BF16_NP)
    b1r = np.ascontiguousarray(np.tile(b1.reshape(1, 2048), (128, 1)))
    bp = np.asarray(b_proj, np.float32)
    ones_c = np.ones((128, 8), BF16_NP)
    w2tb = w2t.astype(BF16_NP)
    wptb = wpt.astype(BF16_NP)

    in_maps = []
    for c in range(N_CORES):
        in_maps.append({
            "xf": np.ascontiguousarray(xf[c * B_PER_CORE:(c + 1) * B_PER_CORE]),
            "w1q": w1q, "w2t": w2tb, "wpt": wptb,
            "b1r": b1r, "b2": b2, "bp": bp,
            "ones_c": ones_c,
        })
    res = bass_utils.run_bass_kernel_spmd(nc, in_maps, core_ids=list(range(N_CORES)))
    _CACHE["last_results"] = res
    y = np.concatenate([np.asarray(res.results[c]["y"], dtype=np.float32)
                        for c in range(N_CORES)], axis=0)
    return np.ascontiguousarray(y.reshape(B, CIN, 32, 32))


# revision 26
# speedup vs baseline: 1.0211x; 1.0016x over previous
"""Trainium2 Bass kernel for nn_Attention_29472065585724.

Reference computation (per batch b of 16, C=1024, H=W=32, seq p2=256, nh=8, hd=512):
    qkv = conv1x1(x, w_qkv, b_qkv)            # [B, 3C, H, W]
    q,k,v = reshape(B, 256, 3, 8, 512) ...    # row-major reshape mixing C and HW
    attn  = softmax(q @ k^T * scale) @ v
    out   = conv1x1(attn_reshaped, w_proj, b_proj)

Strategy (v22, measured 267-272us vs 298us v13 baseline; rel err 5.1e-3):
  - Data-parallel: batch 16 -> 8 cores x 2 batches. No collectives; host gathers.
  - ALL matmul operands bf16 (v13 used f32r for the big GEMMs). Measured on
    this HW: f32r matmuls pay a 227ns cadence per 512-col stream (fp32
    weights load in HI/LO passes, no FWL) while bf16 matmuls run at ~216ns
    (FWL hides the weight load). 1024 big-GEMM matmuls x 11ns = ~11us, and
    accuracy stays ~5e-3 « 2e-2 tolerance. (fp8e4m3 + DoubleRow was
    evaluated and rejected: numpy simulation shows ANY single big GEMM in
    fp8 costs 3.5-4.6e-2 max rel err, over the 2e-2 budget.)
  - DMA issue is the ramp bottleneck, not HBM bandwidth: each DMA_DIRECT2D
    instruction paces with its bytes on the issuing engine (~230GB/s per
    queue). Inputs are split across BOTH hardware DGE queues: w1 quarters
    on the SP (sync) queue, x / b1 / w2 / wp on the Activation (scalar)
    queue, streaming in parallel.
  - GEMM1's first w1-quarter pass is k-outer (8 PSUM banks), consuming the
    (x[k], w1q0[k]) DMA pairs in arrival order; all other GEMM phases are
    k-inner (back-to-back accumulation is ~75ns/matmul faster).
  - b1 host-replicated to [128, 2048] f32, DMA'd in 4 chunks so chunk n
    only gates pass-n evictions (a single 1MB DMA landed ~30us in and
    stalled the PE ~5us).
  - PE warmup: 16 dummy matmuls on a zeroed scratch tile raise the PE
    p-state clock (0.65 -> 2.4GHz after ~3-5us busy) while the first input
    DMAs are in flight.
  - ONE persistent PSUM pool (tag "ps", 8 banks) for every accumulation in
    the program, and all SBUF data pools top-level: bank/slot reuse is an
    exact N-back tile dependency; pool close/reopen barriers (~1us at every
    phase and batch boundary) are gone.
  - x double-buffered across batches so batch 1's input DMAs stream during
    batch 0's compute; batch 1's ramp DMAs are issued before batch 0's
    proj emission so they are not queued behind its y stores.
  - w2/wp stay SBUF-resident across both batches; batch 1 replays batch 0's
    schedule into the same SBUF slots via tile-reuse dependencies.
  - Host-side weight permutation makes every device layout fall out of plain
    GEMMs with zero on-device transposes:
      * q,k produced transposed ([d, seq]) via x-stationary GEMM; softmax
        scale folded into w_q/b_q.
      * v produced in [seq, d]; proj contraction columns permuted so attention
        outputs land contiguously.
  - Softmax without max-subtraction (S bounded ~|6|); denominator via a tiny
    N=8 matmul of exp(S^T) against ones, normalization during PSUM eviction.
  - y stored bf16 and upcast on host. Tail: the program's final proj tile
    runs as two 256-col accumulation groups with split evictions so the
    last y store is 64KB instead of 256KB (~2us off the tail).
"""
import sys

import numpy as np

if "/opt/trn_rl_repo" not in sys.path:
    sys.path.insert(0, "/opt/trn_rl_repo")

import ml_dtypes

import concourse.bass as bass
import concourse.tile as tile
from concourse import bacc, mybir
from concourse import bass_utils

F32 = mybir.dt.float32
BF16 = mybir.dt.bfloat16
AF = mybir.ActivationFunctionType
BF16_NP = ml_dtypes.bfloat16

B_PER_CORE = 2
N_CORES = 8
CIN = 1024
HW = 1024
NH = 8
P2 = 256
HD = 512

_CACHE = {}


def _build_program():
    nc = bacc.Bacc("TRN2", target_bir_lowering=False, debug=False)
    x_d = nc.dram_tensor("xf", [B_PER_CORE, CIN, HW], BF16,
                         kind="ExternalInput").ap()
    w1_d = nc.dram_tensor("w1q", [4, CIN, 512], BF16, kind="ExternalInput").ap()
    w2_d = nc.dram_tensor("w2t", [CIN, 1024], BF16, kind="ExternalInput").ap()
    wp_d = nc.dram_tensor("wpt", [1024, 1024], BF16, kind="ExternalInput").ap()
    b1_d = nc.dram_tensor("b1r", [128, 2048], F32, kind="ExternalInput").ap()
    b2_d = nc.dram_tensor("b2", [1024], F32, kind="ExternalInput").ap()
    bp_d = nc.dram_tensor("bp", [1024], F32, kind="ExternalInput").ap()
    ones_d = nc.dram_tensor("ones_c", [128, 8], BF16, kind="ExternalInput").ap()
    y_d = nc.dram_tensor("y", [B_PER_CORE, 1024, HW], BF16, kind="ExternalOutput").ap()

    with tile.TileContext(nc) as tc:
        with tile.ExitStack() as top:
            persist = top.enter_context(tc.tile_pool(name="persist", bufs=1))
            y_pool = top.enter_context(tc.tile_pool(name="ypool", bufs=4))
            w1_pool = top.enter_context(tc.tile_pool(name="w1pool", bufs=1))
            w2_pool = top.enter_context(tc.tile_pool(name="w2pool", bufs=1))

            # Inputs are split across BOTH hardware DGE queues: the per-queue
            # DMA issue rate is only ~230GB/s (each DMA_DIRECT2D instruction
            # paces with its bytes on the issuing engine), so x / w2 / wp /
            # b1_bc stream on the Activation (scalar) queue while the w1
            # quarters stream on the SP (sync) queue in parallel.
            b2_sb = persist.tile([128, 8], F32, name="b2_sb")
            bp_sb = persist.tile([128, 8], F32, name="bp_sb")
            ones_col = persist.tile([128, 8], BF16, name="ones_col")
            b1_bc = persist.tile([128, 2048], F32, name="b1_bc")

            w2_sb = [w2_pool.tile([128, 1024], BF16, name=f"w2sb{k}", tag=f"w2sb{k}")
                     for k in range(8)]
            wp_pool = top.enter_context(tc.tile_pool(name="wppool", bufs=1))
            wp_sb = [wp_pool.tile([128, 1024], BF16, name=f"wpsb{k}", tag=f"wpsb{k}")
                     for k in range(8)]

            # x double-buffered across batches (persistent tiles): batch 1's
            # input DMAs have no slot-reuse dependency, so they stream during
            # batch 0's compute instead of queueing behind its y stores.
            x_pool = top.enter_context(tc.tile_pool(name="xpool", bufs=1))
            x_sb_all = [[x_pool.tile([128, HW], BF16, name=f"xsb{b}_{k}",
                                     tag=f"xsb{b}_{k}") for k in range(8)]
                        for b in range(B_PER_CORE)]

            # All SBUF data pools are top-level and persistent: batch 1
            # re-allocates the same tags, so cross-batch reuse is enforced by
            # exact tile dependencies instead of pool-close barriers (a pool
            # close/reopen joins on ALL the pool's prior accesses and was
            # costing ~1us at each phase/batch boundary).
            qk_pool = top.enter_context(tc.tile_pool(name="qkpool", bufs=1))
            v_pool = top.enter_context(tc.tile_pool(name="vpool", bufs=1))
            ao_pool = top.enter_context(tc.tile_pool(name="aopool", bufs=1))
            e_pool = top.enter_context(tc.tile_pool(name="epool", bufs=2))
            r_pool = top.enter_context(tc.tile_pool(name="rpool", bufs=4))

            # PE warmup: dummy matmuls on a zeroed scratch tile while the
            # first input DMAs are in flight. Costs nothing (the PE would
            # idle anyway) and raises the PE p-state clock (0.65 -> 2.4GHz
            # after ~3us of continuous execution) before the real pass-0.
            # The warmup PSUM pool releases before ps_pool opens below.
            scratch = persist.tile([128, 256], BF16, name="warm_sb")
            nc.vector.memset(scratch[:], 0.0)
            with tc.tile_pool(name="warmps", bufs=1, space="PSUM") as wps:
                wtile = wps.tile([128, 256], F32, name="warm_ps")
                for _ in range(16):
                    nc.tensor.matmul(wtile[:], scratch[:, 0:128],
                                     scratch[:, 0:256], start=True, stop=True)

            # ONE persistent PSUM pool for every accumulation in the program:
            # all tiles share tag "ps" and rotate through the 8 banks, so
            # bank reuse is an exact 8-allocations-back tile dependency and
            # no PSUM pool is ever closed mid-program.
            ps_pool = top.enter_context(tc.tile_pool(name="pspool", bufs=8,
                                                     space="PSUM"))

            shared = dict(nc=nc, tc=tc, w1_d=w1_d, w2_d=w2_d, wp_d=wp_d,
                          y_d=y_d, w1_pool=w1_pool, w2_sb=w2_sb, wp_sb=wp_sb,
                          b1_bc=b1_bc, b2_sb=b2_sb, bp_sb=bp_sb,
                          ones_col=ones_col, y_pool=y_pool, ps_pool=ps_pool,
                          qk_pool=qk_pool, v_pool=v_pool, ao_pool=ao_pool,
                          e_pool=e_pool, r_pool=r_pool)

            early0 = _issue_early_dmas(nc, 0, x_d, w1_d, x_sb_all[0], w1_pool)
            # behind x on the scalar queue: b1_bc in 4 chunks (chunk n gates
            # only pass-n's eviction; a single 1MB DMA would land ~30us in
            # and stall the PE ~5us), then consts, then w2 (needed ~75us)
            # and wp (~105us)
            for n in range(4):
                nc.scalar.dma_start(b1_bc[:, 512 * n:512 * n + 512],
                                    b1_d[:, 512 * n:512 * n + 512])
            nc.scalar.dma_start(b2_sb[:], b2_d.rearrange("(t p) -> p t", p=128))
            nc.scalar.dma_start(bp_sb[:], bp_d.rearrange("(t p) -> p t", p=128))
            nc.scalar.dma_start(ones_col[:], ones_d[:])
            for k in range(8):
                nc.scalar.dma_start(w2_sb[k][:], w2_d[128 * k:128 * k + 128, :])
            for k in range(8):
                nc.scalar.dma_start(wp_sb[k][:], wp_d[128 * k:128 * k + 128, :])
            ctx0 = _emit_front(shared, 0, x_sb_all[0], early0)
            early1 = _issue_early_dmas(nc, 1, x_d, w1_d, x_sb_all[1], w1_pool)
            _emit_proj(shared, ctx0)
            ctx1 = _emit_front(shared, 1, x_sb_all[1], early1)
            _emit_proj(shared, ctx1)
    nc.compile()
    return nc


def _issue_early_dmas(nc, b, x_d, w1_d, x_sb, w1_pool):
    """Queue batch b's GEMM1 ramp DMAs: x tiles on the scalar queue, w1
    quarters 0-1 on the sync queue - the two streams run in parallel and
    pass 0 consumes (x[k], w1q0[k]) pairs in arrival order."""
    for k in range(8):
        nc.scalar.dma_start(x_sb[k][:], x_d[b, 128 * k:128 * k + 128, :])
    q0 = [w1_pool.tile([128, 512], BF16, name=f"w1q{b}_0_{k}",
                       tag=f"qbuf0_{k}") for k in range(8)]
    for k in range(8):
        nc.sync.dma_start(q0[k][:], w1_d[0, 128 * k:128 * k + 128, :])
    q1 = [w1_pool.tile([128, 512], BF16, name=f"w1q{b}_1_{k}",
                       tag=f"qbuf1_{k}") for k in range(8)]
    for k in range(8):
        nc.sync.dma_start(q1[k][:], w1_d[1, 128 * k:128 * k + 128, :])
    return q0, q1


def _emit_front(shared, b, x_sb, early):
    nc, tc = shared["nc"], shared["tc"]
    w1_d = shared["w1_d"]
    w1_pool, w2_sb = shared["w1_pool"], shared["w2_sb"]
    b1_bc, b2_sb, ones_col = shared["b1_bc"], shared["b2_sb"], shared["ones_col"]
    psp = shared["ps_pool"]
    e_pool, r_pool = shared["e_pool"], shared["r_pool"]

    def load_w1_quarter(n):
        w1q = [w1_pool.tile([128, 512], BF16, name=f"w1q{b}_{n}_{k}",
                            tag=f"qbuf{n % 2}_{k}") for k in range(8)]
        for k in range(8):
            nc.sync.dma_start(w1q[k][:], w1_d[n, 128 * k:128 * k + 128, :])
        return w1q

    qkT = [shared["qk_pool"].tile([128, 2048], BF16, name=f"qkT{b}_{m}",
                                  tag=f"qkT{m}") for m in range(8)]
    v_sb = [shared["v_pool"].tile([128, 1024], BF16, name=f"vsb{b}_{m}",
                                  tag=f"vsb{m}") for m in range(8)]

    # ---------------- QKV GEMMs ----------------
    q0, q1 = early
    w1quads = [q0, q1, load_w1_quarter(2), load_w1_quarter(3)]

    # GEMM1 (q,k): quarter pass 0 k-outer, consuming the (x[k], w1q0[k])
    # DMA pairs in arrival order; passes 1-3 k-inner (back-to-back
    # accumulation is ~75ns/matmul faster than bank-interleaved k-outer)
    pss = [psp.tile([128, 512], F32, name=f"psg1_{b}_0_{m}",
                    tag="ps") for m in range(8)]
    for k in range(8):
        for m in range(8):
            nc.tensor.matmul(
                pss[m][:],
                x_sb[k][:, 128 * m:128 * m + 128],
                w1quads[0][k][:],
                start=(k == 0), stop=(k == 7))
    for m in range(8):
        nc.vector.tensor_add(qkT[m][:, 0:512], pss[m][:],
                             b1_bc[:, 0:512])
    for n in range(1, 4):
        w1q = w1quads[n]
        for m in range(8):
            ps = psp.tile([128, 512], F32, name=f"psg1_{b}_{n}_{m}",
                          tag="ps")
            for k in range(8):
                nc.tensor.matmul(
                    ps[:],
                    x_sb[k][:, 128 * m:128 * m + 128],
                    w1q[k][:],
                    start=(k == 0), stop=(k == 7))
            nc.vector.tensor_add(qkT[m][:, 512 * n:512 * n + 512],
                                 ps[:], b1_bc[:, 512 * n:512 * n + 512])

    # GEMM2 (v): k-inner
    for m in range(8):
        for n in range(2):
            ps = psp.tile([128, 512], F32, name=f"psg2_{b}_{m}_{n}",
                          tag="ps")
            for k in range(8):
                nc.tensor.matmul(
                    ps[:],
                    w2_sb[k][:, 128 * m:128 * m + 128],
                    x_sb[k][:, 512 * n:512 * n + 512],
                    start=(k == 0), stop=(k == 7))
            nc.scalar.activation(v_sb[m][:, 512 * n:512 * n + 512],
                                 ps[:], AF.Identity, bias=b2_sb[:, m:m + 1])

    # ---------------- attention ----------------
    ao_sb = [shared["ao_pool"].tile([128, 1024], BF16, name=f"aosb{b}_{m}",
                                    tag=f"ao{m}") for m in range(8)]

    def attn_st(h):
        g, half = h // 2, h % 2
        base = 4 * half
        es = []
        for kt in range(2):
            ps = psp.tile([128, 256], F32, name=f"ps_st{b}_{h}_{kt}",
                          tag="ps")
            for d in range(4):
                nc.tensor.matmul(
                    ps[:],
                    qkT[base + d][:, (4 + g) * 256 + 128 * kt:
                                  (4 + g) * 256 + 128 * kt + 128],
                    qkT[base + d][:, g * 256:g * 256 + 256],
                    start=(d == 0), stop=(d == 3))
            e = e_pool.tile([128, 256], BF16, name=f"E{b}_{h}_{kt}",
                            tag=f"E{kt}")
            nc.scalar.activation(e[:], ps[:], AF.Exp)
            es.append(e)
        return es

    def attn_pv(h, es):
        g, half = h // 2, h % 2
        for qt in range(2):
            psO = psp.tile([128, 512], F32, name=f"psO{b}_{h}_{qt}", tag="ps")
            psL = psp.tile([128, 8], F32, name=f"psL{b}_{h}_{qt}", tag="ps")
            for kt in range(2):
                nc.tensor.matmul(
                    psO[:], es[kt][:, 128 * qt:128 * qt + 128],
                    v_sb[2 * g + kt][:, 512 * half:512 * half + 512],
                    start=(kt == 0), stop=(kt == 1))
                nc.tensor.matmul(
                    psL[:], es[kt][:, 128 * qt:128 * qt + 128],
                    ones_col[:, 0:8],
                    start=(kt == 0), stop=(kt == 1))
            r = r_pool.tile([128, 1], F32, name=f"r{b}_{h}_{qt}", tag="r")
            nc.vector.reciprocal(r[:], psL[:, 0:1])
            dst = ao_sb[2 * g + qt]
            nc.vector.tensor_scalar_mul(
                dst[:, 512 * half:512 * half + 512], psO[:], r[:])

    es_next = attn_st(0)
    for h in range(NH):
        es_cur = es_next
        es_next = attn_st(h + 1) if h + 1 < NH else None
        attn_pv(h, es_cur)
    return dict(b=b, ao_sb=ao_sb)


def _emit_proj(shared, ctx):
    # ---------------- proj GEMM: k-inner ----------------
    nc = shared["nc"]
    wp_sb, bp_sb = shared["wp_sb"], shared["bp_sb"]
    y_pool, y_d = shared["y_pool"], shared["y_d"]
    psp = shared["ps_pool"]
    b, ao_sb = ctx["b"], ctx["ao_sb"]
    for m in range(8):
        for n in range(2):
            if b == 1 and m == 7 and n == 1:
                # final tile of the program: two 256-col accumulation groups
                # so the first half's store overlaps the second half's
                # matmuls and the very last store is only 64KB
                for c in range(2):
                    ps = psp.tile([128, 256], F32, name=f"pspf_{c}", tag="ps")
                    for k in range(8):
                        nc.tensor.matmul(
                            ps[:],
                            wp_sb[k][:, 896:1024],
                            ao_sb[k][:, 512 + 256 * c:768 + 256 * c],
                            start=(k == 0), stop=(k == 7))
                    y_sb = y_pool.tile([128, 256], BF16, name=f"ysbf_{c}",
                                       tag="ysbf")
                    if c == 0:
                        nc.scalar.activation(y_sb[:], ps[:], AF.Identity,
                                             bias=bp_sb[:, 7:8])
                    else:
                        nc.vector.tensor_scalar_add(y_sb[:], ps[:],
                                                    bp_sb[:, 7:8])
                    nc.sync.dma_start(
                        y_d[b, 896:1024, 512 + 256 * c:768 + 256 * c],
                        y_sb[:])
                continue
            ps = psp.tile([128, 512], F32, name=f"psp{b}_{m}_{n}",
                          tag="ps")
            for k in range(8):
                nc.tensor.matmul(
                    ps[:],
                    wp_sb[k][:, 128 * m:128 * m + 128],
                    ao_sb[k][:, 512 * n:512 * n + 512],
                    start=(k == 0), stop=(k == 7))
            y_sb = y_pool.tile([128, 512], BF16, name=f"ysb{b}_{m}_{n}",
                               tag="ysb")
            if m == 7 and n == 1:
                # tail: split the last eviction across both engines and
                # two DMAs so the final y store starts ~0.7us earlier
                nc.scalar.activation(y_sb[:, 0:256], ps[:, 0:256],
                                     AF.Identity, bias=bp_sb[:, m:m + 1])
                nc.sync.dma_start(
                    y_d[b, 896:1024, 512:768], y_sb[:, 0:256])
                nc.vector.tensor_scalar_add(y_sb[:, 256:512],
                                            ps[:, 256:512],
                                            bp_sb[:, m:m + 1])
                nc.sync.dma_start(
                    y_d[b, 896:1024, 768:1024], y_sb[:, 256:512])
            else:
                if (2 * m + n) % 2 == 0:
                    nc.scalar.activation(y_sb[:], ps[:], AF.Identity,
                                         bias=bp_sb[:, m:m + 1])
                else:
                    nc.vector.tensor_scalar_add(y_sb[:], ps[:],
                                                bp_sb[:, m:m + 1])
                nc.sync.dma_start(
                    y_d[b, 128 * m:128 * m + 128, 512 * n:512 * n + 512],
                    y_sb[:])


def _prepare_host_inputs(w_qkv, b_qkv, w_proj):
    """Permute weights so device layouts need no transposes. See layout notes."""
    C = CIN
    scale = np.float32((C // NH) ** -0.5)
    g_i, p_i = np.meshgrid(np.arange(4), np.arange(256), indexing="ij")
    # GEMM1 columns: (t, g, p) -> channel 12p + 4t + g
    t_i, g2_i, p2_i = np.meshgrid(np.arange(2), np.arange(4), np.arange(256),
                                  indexing="ij")
    src1 = (12 * p2_i + 4 * t_i + g2_i).reshape(-1)
    w1 = w_qkv[src1, :].astype(np.float32).copy()
    b1 = b_qkv[src1].astype(np.float32).copy()
    w1[:1024] *= scale
    b1[:1024] *= scale
    w1t = np.ascontiguousarray(w1.T)                       # [1024, 2048]
    # GEMM2 rows: r = g*256 + p -> channel 12p + 8 + g
    src2 = (12 * p_i + 8 + g_i).reshape(-1)
    w2t = np.ascontiguousarray(w_qkv[src2, :].T.astype(np.float32))   # [1024, 1024]
    b2 = b_qkv[src2].astype(np.float32).copy()
    # proj contraction: c' = g*256 + p -> orig col 4p + g
    srcp = (4 * p_i + g_i).reshape(-1)
    wpt = np.ascontiguousarray(w_proj[:, srcp].T.astype(np.float32))  # [1024, 1024]
    return w1t, b1, w2t, b2, wpt


def kernel(x, w_qkv, b_qkv, w_proj, b_proj):
    if "nc" not in _CACHE:
        _CACHE["nc"] = _build_program()
    nc = _CACHE["nc"]

    x = np.asarray(x, dtype=np.float32)
    B = x.shape[0]
    xf = x.reshape(B, CIN, HW).astype(BF16_NP)
    w1t, b1, w2t, b2, wpt = _prepare_host_inputs(
        np.asarray(w_qkv, np.float32), np.asarray(b_qkv, np.float32),
        np.asarray(w_proj, np.float32))
    # w1 as four contiguous quarter tensors [4, 1024, 512]
    w1q = np.ascontiguousarray(
        w1t.reshape(CIN, 4, 512).transpose(1, 0, 2)).astype(BF16_NP)
    b1r = np.ascontiguousarray(np.tile(b1.reshape(1, 2048), (128, 1)))
    bp = np.asarray(b_proj, np.float32)
    ones_c = np.ones((128, 8), BF16_NP)
    w2tb = w2t.astype(BF16_NP)
    wptb = wpt.astype(BF16_NP)

    in_maps = []
    for c in range(N_CORES):
        in_maps.append({
            "xf": np.ascontiguousarray(xf[c * B_PER_CORE:(c + 1) * B_PER_CORE]),
            "w1q": w1q, "w2t": w2tb, "wpt": wptb,
            "b1r": b1r, "b2": b2, "bp": bp,
            "ones_c": ones_c,
        })
    res = bass_utils.run_bass_kernel_spmd(nc, in_maps, core_ids=list(range(N_CORES)))
    _CACHE["last_results"] = res
    y = np.concatenate([np.asarray(res.results[c]["y"], dtype=np.float32)
                        for c in range(N_CORES)], axis=0)
    return np.ascontiguousarray(y.reshape(B, CIN, 32, 32))
